# revision 15
# baseline (speedup 1.0000x reference)
# Layer-pipelined Trainium2 Bass kernel for the Tacotron-style decoder.
# 8 cores = 4 pairs. Pair q = (core q, core q+4) handles batch quarter q
# (8 sequences). Core q runs the layer-0 recurrence; core q+4 runs layer-1,
# one 100-step chunk behind, fed by a pairwise AllGather of h chunks.
# All cores execute the SAME program; layer roles differ only in the weight
# DATA (the inactive layer's weights are zeros, making its stages no-ops).
#
# Iteration it in 0..10 (NCH=10 chunks of 100 steps):
#   GEMM   (it>0): xgB[it] = wihL @ hx(=partner h chunk it-1) + bL
#   RECUR  : chunk it of the local-layer recurrence on xgA[min(it,9)] (+xgB[it])
#            (at it==1, state is multiplied by maskcol: 1 on L0 cores, 0 on L1,
#             so L1's real chunk 0 starts from zero state)
#   STORE  : h chunk -> hT_it;  AllGather pair-exchange -> hx
#   PROJ   (fused at end): out chunk c reads hT_{c+1} (L1's real chunk c).
# Host uses only cores 4-7's outputs.
import functools
import numpy as np
import ml_dtypes

B, T, A, M = 32, 1000, 512, 80
P, H = 256, 1024
NCORES = 8
BC = 8                      # sequences per pair
F = BC * T                  # 8000 frames per core, frame f = t*BC + b
G4 = 4 * H
NBLK = H // 128
SBLK = 20                   # recurrence steps per hardware-loop iteration
GORDER = (0, 1, 3, 2)       # on-chip gate order i, f, o, g
NCHUNK = 16                 # frame chunks for batched GEMMs (500 frames)
FCH = F // NCHUNK
NCH = 10                    # pipeline chunks
CT = T // NCH               # 100 steps per chunk
CF = CT * BC                # 800 frames per chunk
NIT = NCH + 1


def _arrange_cols(wt):
    cols = []
    for blk in range(NBLK):
        for go in GORDER:
            cols.append(wt[:, go * H + blk * 128: go * H + (blk + 1) * 128])
    return np.ascontiguousarray(np.concatenate(cols, axis=1))


def _arrange_vec(b):
    return _arrange_cols(b.reshape(1, G4))[0]


@functools.lru_cache(maxsize=None)
def _build(rec_reps=1):
    import concourse.bacc as bacc
    import concourse.mybir as mybir
    from concourse import tile
    import concourse.bass as bass

    dt = mybir.dt
    nc = bacc.Bacc(None)

    # ---------------- I/O ----------------
    mem_f = nc.declare_dram_parameter("mem_f", [F, A], dt.float32, isOutput=False)
    y_f = nc.declare_dram_parameter("y_f", [F, M], dt.float32, isOutput=False)
    ident = nc.declare_dram_parameter("ident", [128, 128], dt.float32, isOutput=False)
    w1t = nc.declare_dram_parameter("w1t", [M, P], dt.float32, isOutput=False)
    w2t = nc.declare_dram_parameter("w2t", [P, P], dt.float32, isOutput=False)
    wih0t = nc.declare_dram_parameter("wih0t", [P + A, G4], dt.bfloat16, isOutput=False)
    whht = nc.declare_dram_parameter("whht", [H, G4], dt.float8e4, isOutput=False)
    wiht = nc.declare_dram_parameter("wiht", [H, G4], dt.bfloat16, isOutput=False)
    b0in = nc.declare_dram_parameter("b0in", [1, G4], dt.float32, isOutput=False)
    bLin = nc.declare_dram_parameter("bLin", [1, G4], dt.float32, isOutput=False)
    wpt_h = nc.declare_dram_parameter("wpt_h", [H, M], dt.bfloat16, isOutput=False)
    wpt_m = nc.declare_dram_parameter("wpt_m", [A, M], dt.bfloat16, isOutput=False)
    bpin = nc.declare_dram_parameter("bpin", [1, M], dt.float32, isOutput=False)
    maskin = nc.declare_dram_parameter("maskin", [128, 1], dt.float32, isOutput=False)
    outT = nc.declare_dram_parameter("outT", [M, F], dt.float32, isOutput=True)

    # ---------------- internal DRAM ----------------
    memT_d = nc.dram_tensor("memT_d", [A, F], dt.bfloat16)
    xgAT = nc.dram_tensor("xgAT", [G4, F], dt.float32)
    xgB = [None] + [nc.dram_tensor(f"xgB{i}", [G4, CF], dt.float32)
                    for i in range(1, NIT)]
    hT = [nc.dram_tensor(f"hT{i}", [H, CF], dt.bfloat16) for i in range(NIT)]
    hx_d = nc.dram_tensor("hx_d", [2 * H, CF], dt.bfloat16)

    FT = (F + 127) // 128

    def ftrows(ft):
        return min(128, F - ft * 128)

    ACT = mybir.ActivationFunctionType
    RG = [[q, q + 4] for q in range(4)]

    with tile.TileContext(nc) as tc:
        with tc.tile_pool(name="const", bufs=1) as cpool:
            idsb = cpool.tile([128, 128], dt.float32, name="idsb")
            nc.sync.dma_start(idsb[:], ident[:])
            b0sb = cpool.tile([128, 32], dt.float32, name="b0sb")
            bLsb = cpool.tile([128, 32], dt.float32, name="bLsb")
            bpsb = cpool.tile([M, 1], dt.float32, name="bpsb")
            masksb = cpool.tile([128, 1], dt.float32, name="masksb")
            nc.sync.dma_start(b0sb[:], b0in[:].rearrange("o (m p) -> (o p) m", p=128))
            nc.sync.dma_start(bLsb[:], bLin[:].rearrange("o (m p) -> (o p) m", p=128))
            nc.sync.dma_start(bpsb[:], bpin[:].rearrange("o (m u) -> (o m) u", u=1))
            nc.sync.dma_start(masksb[:], maskin[:])

            # persistent channel-major activations
            with tc.tile_pool(name="actsb", bufs=1) as apool:
                prevT = apool.tile([M, F], dt.float32, name="prevT")
                p2T = apool.tile([128, 2 * F], dt.bfloat16, name="p2T")

                # ---------- Ph1: transposes ----------
                with tc.tile_pool(name="tr", bufs=3) as trp, \
                     tc.tile_pool(name="trps", bufs=2, space="PSUM") as trps:
                    for ft in range(FT):
                        r = ftrows(ft)
                        yin = trp.tile([128, M], dt.float32, name="yin", tag="yin")
                        if ft == 0:
                            nc.gpsimd.memset(yin[:, :], 0.0)
                            nc.sync.dma_start(yin[BC:r, :], y_f[0:r - BC, :])
                        else:
                            nc.sync.dma_start(yin[0:r, :], y_f[ft * 128 - BC: ft * 128 - BC + r, :])
                        yps = trps.tile([M, 128], dt.float32, name="yps", tag="yps")
                        nc.tensor.transpose(yps[:, 0:r], yin[0:r, :], idsb[0:r, 0:r])
                        nc.scalar.copy(prevT[:, ft * 128: ft * 128 + r], yps[:, 0:r])
                        for cb in range(A // 128):
                            min_ = trp.tile([128, 128], dt.float32, name="min_", tag="min")
                            nc.sync.dma_start(min_[0:r, :], mem_f[ft * 128: ft * 128 + r, cb * 128:(cb + 1) * 128])
                            mps = trps.tile([128, 128], dt.float32, name="mps", tag="mps")
                            nc.tensor.transpose(mps[:, 0:r], min_[0:r, :], idsb[0:r, 0:r])
                            mrow = trp.tile([128, 128], dt.bfloat16, name="mrow", tag="mrow")
                            nc.scalar.copy(mrow[:, 0:r], mps[:, 0:r])
                            nc.sync.dma_start(memT_d[cb * 128:(cb + 1) * 128, ft * 128: ft * 128 + r], mrow[:, 0:r])

                # ---------- Ph2: prenet ----------
                with tc.tile_pool(name="pn", bufs=2) as pnp, \
                     tc.tile_pool(name="pnps", bufs=2, space="PSUM") as pnps:
                    w1sb = pnp.tile([M, P], dt.float32, name="w1sb")
                    nc.sync.dma_start(w1sb[:], w1t[:])
                    p1T = pnp.tile([128, 2 * F], dt.float32, name="p1T")
                    for m in range(P // 128):
                        for n in range(NCHUNK):
                            ps = pnps.tile([128, FCH], dt.float32, name="pnps1", tag=f"pn{n % 4}")
                            nc.tensor.matmul(ps[:], w1sb[:, m * 128:(m + 1) * 128],
                                             prevT[:, n * FCH:(n + 1) * FCH], start=True, stop=True)
                            nc.scalar.activation(p1T[:, m * F + n * FCH: m * F + (n + 1) * FCH], ps[:], ACT.Relu)
                    w2sb = pnp.tile([128, 2 * P], dt.float32, name="w2sb")
                    for k in range(P // 128):
                        nc.sync.dma_start(w2sb[:, k * P:(k + 1) * P], w2t[k * 128:(k + 1) * 128, :])
                    for m in range(P // 128):
                        for n in range(NCHUNK):
                            ps = pnps.tile([128, FCH], dt.float32, name="pnps2", tag=f"pn{n % 4}")
                            for k in range(P // 128):
                                nc.tensor.matmul(ps[:], w2sb[:, k * P + m * 128: k * P + (m + 1) * 128],
                                                 p1T[:, k * F + n * FCH: k * F + (n + 1) * FCH],
                                                 start=(k == 0), stop=(k == 1))
                            nc.scalar.activation(p2T[:, m * F + n * FCH: m * F + (n + 1) * FCH], ps[:], ACT.Relu)

                # ---------- Ph3: xgA = wih0 @ x + b0 ----------
                KX = 6
                with tc.tile_pool(name="x0", bufs=2) as x0p, \
                     tc.tile_pool(name="x0ps", bufs=1, space="PSUM") as x0ps:
                    memTsb = x0p.tile([128, 4 * F], dt.bfloat16, name="memTsb")
                    for cb in range(4):
                        nc.sync.dma_start(memTsb[:, cb * F:(cb + 1) * F], memT_d[cb * 128:(cb + 1) * 128, :])

                    def x_rhs(k, n):
                        if k < 2:
                            return p2T[:, k * F + n * FCH: k * F + (n + 1) * FCH]
                        cb = k - 2
                        return memTsb[:, cb * F + n * FCH: cb * F + n * FCH + FCH]

                    for half in range(2):
                        for m in range(32):
                            wtile = x0p.tile([128, 6 * 128], dt.bfloat16, name="wtile", tag="w0t")
                            for k in range(KX):
                                nc.sync.dma_start(
                                    wtile[:, k * 128:(k + 1) * 128],
                                    wih0t[k * 128:(k + 1) * 128, m * 128:(m + 1) * 128])
                            pss = []
                            for nn in range(NCHUNK // 2):
                                ps = x0ps.tile([128, FCH], dt.float32, name="x0psn", tag=f"x0{nn}")
                                pss.append(ps)
                            for k in range(KX):
                                for nn in range(NCHUNK // 2):
                                    n = half * (NCHUNK // 2) + nn
                                    nc.tensor.matmul(pss[nn][:], wtile[:, k * 128:(k + 1) * 128], x_rhs(k, n),
                                                     start=(k == 0), stop=(k == KX - 1))
                            for nn in range(NCHUNK // 2):
                                n = half * (NCHUNK // 2) + nn
                                otile = x0p.tile([128, FCH], dt.float32, name="otile", tag="x0o")
                                nc.vector.tensor_scalar_add(otile[:], pss[nn][:], b0sb[:, m:m + 1])
                                nc.sync.dma_start(xgAT[m * 128:(m + 1) * 128, n * FCH:(n + 1) * FCH], otile[:])

            # ---------- pipeline loop ----------
            with tc.tile_pool(name="rcper", bufs=1) as rp:
                whsb = rp.tile([128, 8 * G4], dt.float8e4, name="whsb")
                for k in range(8):
                    nc.sync.dma_start(whsb[:, k * G4:(k + 1) * G4], whht[k * 128:(k + 1) * 128, :])
                hbuf = [rp.tile([128, BC * NBLK], dt.bfloat16, name=f"hbuf{i}") for i in range(2)]
                cbuf = [rp.tile([128, BC * NBLK], dt.float32, name=f"cbuf{i}") for i in range(2)]
                nc.gpsimd.memset(hbuf[0][:], 0.0)
                nc.gpsimd.memset(cbuf[0][:], 0.0)

                def gemm_stage(it):
                    # xgB[it] = wihL @ hx[0:1024] + bL   (2 n-chunks of 400)
                    with tc.tile_pool(name=f"g{it}", bufs=2) as gp, \
                         tc.tile_pool(name=f"gps{it}", bufs=2, space="PSUM") as gps:
                        hxsb = gp.tile([128, 8 * CF], dt.bfloat16, name="hxsb")
                        for k in range(8):
                            nc.sync.dma_start(hxsb[:, k * CF:(k + 1) * CF],
                                              hx_d[k * 128:(k + 1) * 128, :])
                        for m in range(32):
                            wtile = gp.tile([128, 8 * 128], dt.bfloat16, name="gwt", tag="gwt")
                            for k in range(8):
                                nc.sync.dma_start(
                                    wtile[:, k * 128:(k + 1) * 128],
                                    wiht[k * 128:(k + 1) * 128, m * 128:(m + 1) * 128])
                            for n in range(2):
                                ps = gps.tile([128, CF // 2], dt.float32, name="gpsn", tag=f"g{n}")
                                for k in range(8):
                                    nc.tensor.matmul(
                                        ps[:], wtile[:, k * 128:(k + 1) * 128],
                                        hxsb[:, k * CF + n * (CF // 2): k * CF + (n + 1) * (CF // 2)],
                                        start=(k == 0), stop=(k == 7))
                                ot = gp.tile([128, CF // 2], dt.float32, name="got", tag="got")
                                nc.vector.tensor_scalar_add(ot[:], ps[:], bLsb[:, m:m + 1])
                                nc.sync.dma_start(
                                    xgB[it][m * 128:(m + 1) * 128, n * (CF // 2):(n + 1) * (CF // 2)],
                                    ot[:])

                def recurrence_chunk(it):
                    xgA_off = min(it, NCH - 1) * CF
                    NB = CT // SBLK  # 25 blocks
                    W = BC * SBLK    # 32 frames per block
                    with tc.tile_pool(name=f"rcx{it}", bufs=2) as rxp, \
                         tc.tile_pool(name=f"rcps{it}", bufs=1, space="PSUM") as rps, \
                         tc.tile_pool(name=f"rct{it}", bufs=2) as rtp:
                        psl = [rps.tile([128, 4 * BC], dt.float32, name=f"ps{blk}", tag=f"ps{blk}")
                               for blk in range(NBLK)]

                        with tc.For_i(0, NB, 1, hint_engines=(mybir.EngineType.PE,
                                                              mybir.EngineType.DVE,
                                                              mybir.EngineType.Activation)) as bi:
                            xgsb = rxp.tile([128, 32 * W], dt.float32, name="xgsb", tag="xgsb")
                            for rr in range(32):
                                nc.sync.dma_start(
                                    xgsb[:, rr * W:(rr + 1) * W],
                                    xgAT[rr * 128:(rr + 1) * 128,
                                         bass.ds(bi * W + xgA_off, W)])
                            if it > 0:
                                xgsbB = rxp.tile([128, 32 * W], dt.float32, name="xgsbB", tag="xgsbB")
                                for rr in range(32):
                                    nc.sync.dma_start(
                                        xgsbB[:, rr * W:(rr + 1) * W],
                                        xgB[it][rr * 128:(rr + 1) * 128, bass.ts(bi, W)])
                            hblk = rxp.tile([128, NBLK * W], dt.bfloat16, name="hblk", tag="hblk")
                            for s in range(SBLK):
                                pin, pout = s % 2, 1 - (s % 2)
                                h_in, h_out = hbuf[pin], hbuf[pout]
                                c_in, c_out = cbuf[pin], cbuf[pout]
                                for blk in range(NBLK):
                                    ps = psl[blk]
                                    for gi in range(4):
                                        mm = blk * 4 + gi
                                        for k in range(8):
                                            nc.tensor.matmul(
                                                ps[:, gi * BC:(gi + 1) * BC],
                                                whsb[:, k * G4 + mm * 128: k * G4 + (mm + 1) * 128],
                                                h_in[:, k * BC:(k + 1) * BC],
                                                start=(k == 0), stop=(k == 7))
                                    zt = rtp.tile([128, 4 * BC], dt.float32, name="zt", tag=f"zt{blk % 4}")
                                    xga = xgsb[:].rearrange("p (r c) -> p r c", r=32)[
                                        :, blk * 4: blk * 4 + 4, s * BC: s * BC + BC]
                                    psa = ps[:].rearrange("p (r c) -> p r c", r=4)
                                    zta = zt[:].rearrange("p (r c) -> p r c", r=4)
                                    nc.vector.tensor_add(zta, psa, xga)
                                    if it > 0:
                                        xgb = xgsbB[:].rearrange("p (r c) -> p r c", r=32)[
                                            :, blk * 4: blk * 4 + 4, s * BC: s * BC + BC]
                                        nc.vector.tensor_add(zta, zta, xgb)
                                    st = rtp.tile([128, 3 * BC], dt.float32, name="st", tag=f"st{blk % 4}")
                                    nc.scalar.activation(st[:], zt[:, 0:3 * BC], ACT.Sigmoid)
                                    gt = rtp.tile([128, BC], dt.float32, name="gt", tag=f"gt{blk % 4}")
                                    nc.scalar.activation(gt[:], zt[:, 3 * BC:4 * BC], ACT.Tanh)
                                    aa = rtp.tile([128, BC], dt.float32, name="aa", tag=f"aa{blk % 4}")
                                    nc.vector.tensor_mul(aa[:], st[:, BC:2 * BC], c_in[:, blk * BC:(blk + 1) * BC])
                                    bb = rtp.tile([128, BC], dt.float32, name="bb", tag=f"bb{blk % 4}")
                                    nc.vector.tensor_mul(bb[:], st[:, 0:BC], gt[:])
                                    nc.vector.tensor_add(c_out[:, blk * BC:(blk + 1) * BC], aa[:], bb[:])
                                    tcx = rtp.tile([128, BC], dt.float32, name="tcx", tag=f"tc{blk % 4}")
                                    nc.scalar.activation(tcx[:], c_out[:, blk * BC:(blk + 1) * BC], ACT.Tanh)
                                    nc.vector.tensor_mul(h_out[:, blk * BC:(blk + 1) * BC], st[:, 2 * BC:3 * BC], tcx[:])
                                    nc.vector.tensor_copy(
                                        hblk[:, blk * W + s * BC: blk * W + (s + 1) * BC],
                                        h_out[:, blk * BC:(blk + 1) * BC])
                            for blk in range(NBLK):
                                nc.sync.dma_start(
                                    hT[it][blk * 128:(blk + 1) * 128, bass.ts(bi, W)],
                                    hblk[:, blk * W:(blk + 1) * W])

                import concourse.mybir as mybir2
                for it in range(NIT):
                    if it == 1:
                        # zero the carried state on L1 cores (mask=0), keep on L0
                        nc.vector.tensor_scalar_mul(hbuf[0][:], hbuf[0][:], masksb[:, 0:1])
                        nc.vector.tensor_scalar_mul(cbuf[0][:], cbuf[0][:], masksb[:, 0:1])
                    if it > 0:
                        gemm_stage(it)
                    for _ in range(rec_reps):
                        recurrence_chunk(it)
                    if it < NIT - 1:
                        nc.gpsimd.collective_compute(
                            "AllGather",
                            mybir.AluOpType.bypass,
                            ins=[hT[it][:, :]],
                            outs=[hx_d[:, :]],
                            replica_groups=RG,
                        )

            # ---------- projection ----------
            with tc.tile_pool(name="pj", bufs=2) as pjp, \
                 tc.tile_pool(name="pjps", bufs=2, space="PSUM") as pjps:
                wphsb = pjp.tile([128, 8 * M], dt.bfloat16, name="wphsb")
                for k in range(8):
                    nc.sync.dma_start(wphsb[:, k * M:(k + 1) * M], wpt_h[k * 128:(k + 1) * 128, :])
                wpmsb = pjp.tile([128, 4 * M], dt.bfloat16, name="wpmsb")
                for k in range(4):
                    nc.sync.dma_start(wpmsb[:, k * M:(k + 1) * M], wpt_m[k * 128:(k + 1) * 128, :])
                NPC = CF // 400  # 2 n-chunks of 400 per chunk
                for c in range(NCH):
                    for n in range(NPC):
                        f0 = n * 400
                        h1sb = pjp.tile([128, 8 * 400], dt.bfloat16, name="h1sb", tag="h1sb")
                        for k in range(8):
                            nc.sync.dma_start(h1sb[:, k * 400:(k + 1) * 400],
                                              hT[c + 1][k * 128:(k + 1) * 128, f0:f0 + 400])
                        msb = pjp.tile([128, 4 * 400], dt.bfloat16, name="msb", tag="msb")
                        for k in range(4):
                            nc.sync.dma_start(msb[:, k * 400:(k + 1) * 400],
                                              memT_d[k * 128:(k + 1) * 128, c * CF + f0: c * CF + f0 + 400])
                        ps = pjps.tile([M, 400], dt.float32, name="pjpsn", tag=f"pj{n % 2}")
                        for k in range(8):
                            nc.tensor.matmul(ps[:], wphsb[:, k * M:(k + 1) * M],
                                             h1sb[:, k * 400:(k + 1) * 400],
                                             start=(k == 0), stop=False)
                        for cb in range(4):
                            nc.tensor.matmul(ps[:], wpmsb[:, cb * M:(cb + 1) * M],
                                             msb[:, cb * 400:(cb + 1) * 400],
                                             start=False, stop=(cb == 3))
                        otile = pjp.tile([M, 400], dt.float32, name="pjo", tag="pjo")
                        nc.vector.tensor_scalar_add(otile[:], ps[:], bpsb[:, 0:1])
                        nc.sync.dma_start(outT[:, c * CF + f0: c * CF + f0 + 400], otile[:])

    nc.finalize()
    return nc


def prepare_in_maps(memory, y_mels, W1, W2, w_ih0, w_hh0, b_ih0, b_hh0,
                    w_ih1, w_hh1, b_ih1, b_hh1, W_proj, b_proj):
    bf16 = ml_dtypes.bfloat16
    fp8 = ml_dtypes.float8_e4m3
    ident = np.eye(128, dtype=np.float32)
    w1t_r = np.ascontiguousarray(W1.T.astype(np.float32))
    w2t_r = np.ascontiguousarray(W2.T.astype(np.float32))
    wih0_r = _arrange_cols(w_ih0.T.astype(np.float32)).astype(bf16)
    whh0_r = _arrange_cols(w_hh0.T.astype(np.float32)).astype(fp8)
    wih1_r = _arrange_cols(w_ih1.T.astype(np.float32)).astype(bf16)
    whh1_r = _arrange_cols(w_hh1.T.astype(np.float32)).astype(fp8)
    b0_r = _arrange_vec((b_ih0 + b_hh0).astype(np.float32)).reshape(1, G4)
    b1_r = _arrange_vec((b_ih1 + b_hh1).astype(np.float32)).reshape(1, G4)
    wpt = W_proj.T.astype(np.float32)
    wpt_h = np.ascontiguousarray(wpt[:H]).astype(bf16)
    wpt_m = np.ascontiguousarray(wpt[H:]).astype(bf16)
    bp = b_proj.astype(np.float32).reshape(1, M)

    z_w1t = np.zeros_like(w1t_r)
    z_w2t = np.zeros_like(w2t_r)
    z_wih0 = np.zeros_like(wih0_r)
    z_b = np.zeros_like(b0_r)
    z_wih1 = np.zeros_like(wih1_r)

    in_maps = []
    for c in range(NCORES):
        is_l0 = c < 4
        q = c % 4
        mem_c = memory[q * BC:(q + 1) * BC]
        y_c = y_mels[q * BC:(q + 1) * BC]
        mem_fc = np.ascontiguousarray(mem_c.transpose(1, 0, 2).reshape(F, A)).astype(np.float32)
        y_fc = np.ascontiguousarray(y_c.transpose(1, 0, 2).reshape(F, M)).astype(np.float32)
        mask = np.full((128, 1), 1.0 if is_l0 else 0.0, np.float32)
        in_maps.append(dict(
            mem_f=mem_fc, y_f=y_fc, ident=ident,
            w1t=w1t_r if is_l0 else z_w1t,
            w2t=w2t_r if is_l0 else z_w2t,
            wih0t=wih0_r if is_l0 else z_wih0,
            whht=whh0_r if is_l0 else whh1_r,
            wiht=z_wih1 if is_l0 else wih1_r,
            b0in=b0_r if is_l0 else z_b,
            bLin=z_b if is_l0 else b1_r,
            wpt_h=wpt_h, wpt_m=wpt_m, bpin=bp, maskin=mask))
    return in_maps


def assemble_output(results):
    outs = []
    for q in range(4):
        oT = results[q + 4]["outT"]                    # [80, 8000]
        outs.append(oT.reshape(M, T, BC).transpose(2, 1, 0))
    return np.concatenate(outs, axis=0).astype(np.float32)


def kernel(memory, y_mels, W1, W2, w_ih0, w_hh0, b_ih0, b_hh0,
           w_ih1, w_hh1, b_ih1, b_hh1, W_proj, b_proj, _trace=False):
    from concourse.bass_utils import run_bass_kernel_spmd

    nc = _build()
    in_maps = prepare_in_maps(
        memory, y_mels, W1, W2, w_ih0, w_hh0, b_ih0, b_hh0,
        w_ih1, w_hh1, b_ih1, b_hh1, W_proj, b_proj)
    res = run_bass_kernel_spmd(nc, in_maps, core_ids=list(range(NCORES)))
    full = assemble_output(res.results)
    kernel.last_exec_time_ns = res.exec_time_ns
    return full
